# revision 5
# baseline (speedup 1.0000x reference)
"""Trainium2 Bass kernel for nn_AttentionModel (sparse banded attention).

Math (per batch element, data-parallel over 8 cores):
  qs    = q @ W_score.T
  score = qs @ k.T                      # only the 129-wide causal band matters
  w     = banded_softmax(score)         # full-row max cancels mathematically
  c     = w @ k
  enh   = tanh(concat([c, q]) @ W_enh.T + b_enh)
  out   = sigmoid(enh @ W_mask.T + b_mask)

Implementation notes:
  - T=2000 padded: keys get 128 zero rows in front + 48 tail -> 2176 = 17*128;
    queries get 48 tail pad -> 2048 = 16*128.  Query tile j attends key blocks
    j (prev) and j+1 (diag) of the padded key array.
  - Band masking is done by accumulating an additive -30000 mask into the score
    PSUM tile with an identity-weight matmul (PE is cheaper than DVE here).
  - Softmax skips the max subtraction (it cancels exactly; in-band |score|<~60
    so exp() is safe in fp32).  exp runs on ACT with accum_out giving the row
    sums for free; normalization is a per-partition tensor_scalar multiply.
  - sigmoid(x) is computed as 0.5*tanh(0.5x)+0.5 so the whole kernel uses the
    single "exp_and_others" ACT table set (exp+tanh) -> no table reloads.
  - All weights are replicated; everything is fp32.
"""

import numpy as np
from contextlib import ExitStack

import concourse.bass as bass
import concourse.bacc as bacc
import concourse.tile as tile
from concourse import mybir
from concourse.bass_utils import run_bass_kernel_spmd

F32 = mybir.dt.float32
AF = mybir.ActivationFunctionType
ALU = mybir.AluOpType

B, T, H, F_OUT = 8, 2000, 256, 257
TPK = 2176   # padded key length   (128 front + 2000 + 48 tail)
TPQ = 2048   # padded query length (2000 + 48 tail)
NT = 16      # query tiles of 128
NKB = 17     # key blocks of 128
NEG = -30000.0
N_CORES = 8

_CACHE = {}


def _consts():
    t_i = np.arange(128, dtype=np.int32)[:, None]
    s_i = np.arange(128, dtype=np.int32)[None, :]
    mask_prev = np.where(s_i >= t_i, 0.0, NEG).astype(np.float32)
    mask_diag = np.where(s_i <= t_i, 0.0, NEG).astype(np.float32)
    mask_std = np.ascontiguousarray(np.concatenate([mask_prev, mask_diag], 1))
    mask_t0 = np.ascontiguousarray(
        np.concatenate([np.full((128, 128), NEG, np.float32), mask_diag], 1)
    )
    ident = np.eye(128, dtype=np.float32)
    ones_row = np.ones((1, 128), dtype=np.float32)
    return ident, mask_std, mask_t0, ones_row


def build_nc():
    nc = bacc.Bacc("TRN2", target_bir_lowering=False, debug=False,
                   num_devices=N_CORES)

    kT = nc.declare_dram_parameter("kT", [H, TPK], F32, isOutput=False)
    kN = nc.declare_dram_parameter("kN", [TPK, H], F32, isOutput=False)
    qT = nc.declare_dram_parameter("qT", [H, TPQ], F32, isOutput=False)
    WsT = nc.declare_dram_parameter("WsT", [H, H], F32, isOutput=False)
    WeT = nc.declare_dram_parameter("WeT", [2 * H, H], F32, isOutput=False)
    WmT = nc.declare_dram_parameter("WmT", [H, F_OUT], F32, isOutput=False)
    be = nc.declare_dram_parameter("be", [H, 1], F32, isOutput=False)
    bm = nc.declare_dram_parameter("bm", [1, F_OUT], F32, isOutput=False)
    out = nc.declare_dram_parameter("out", [T, F_OUT], F32, isOutput=True)

    ident_np, mask_std_np, mask_t0_np, ones_np = _consts()
    ident_d = nc.inline_tensor(ident_np, "identc")
    mask_std_d = nc.inline_tensor(mask_std_np, "mask_stdc")
    mask_t0_d = nc.inline_tensor(mask_t0_np, "mask_t0c")
    ones_d = nc.inline_tensor(ones_np, "onesc")

    with tile.TileContext(nc) as tc, ExitStack() as ctx:
        const = ctx.enter_context(tc.tile_pool(name="const", bufs=1))
        io = ctx.enter_context(tc.tile_pool(name="io", bufs=1))
        wk = ctx.enter_context(tc.tile_pool(name="wk", bufs=3))
        stat = ctx.enter_context(tc.tile_pool(name="stat", bufs=4))
        pmm = ctx.enter_context(tc.tile_pool(name="pmm", bufs=4, space="PSUM"))
        pwt = ctx.enter_context(tc.tile_pool(name="pwt", bufs=2, space="PSUM"))
        pct = ctx.enter_context(tc.tile_pool(name="pct", bufs=2, space="PSUM"))

        def cload(tag, shape, src):
            t = const.tile(shape, F32, tag=tag, name=tag)
            nc.sync.dma_start(t[:], src)
            return t

        ident = cload("ident", [128, 128], ident_d[:])
        mask_std = cload("mask_std", [128, 256], mask_std_d[:])
        mask_t0 = cload("mask_t0", [128, 256], mask_t0_d[:])
        ones_t = cload("ones", [1, 128], ones_d[:])
        bm_t = cload("bm", [1, F_OUT], bm[:])
        wst = [cload(f"wst{c}", [128, H], WsT[c * 128:(c + 1) * 128, :])
               for c in range(2)]
        wet = [cload(f"wet{d}", [128, H], WeT[d * 128:(d + 1) * 128, :])
               for d in range(4)]
        wmt = [cload(f"wmt{f}", [128, F_OUT], WmT[f * 128:(f + 1) * 128, :])
               for f in range(2)]
        bet = [cload(f"bet{f}", [128, 1], be[f * 128:(f + 1) * 128, :])
               for f in range(2)]

        # ---- big persistent SBUF buffers ----
        kT_t = []
        for c in range(2):
            t = io.tile([128, TPK], F32, tag=f"kT{c}", name=f"kT{c}")
            nc.sync.dma_start(t[:], kT[c * 128:(c + 1) * 128, :])
            kT_t.append(t)
        qT_t = []
        for c in range(2):
            t = io.tile([128, TPQ], F32, tag=f"qT{c}", name=f"qT{c}")
            for nb in range(4):
                nc.sync.dma_start(t[:, nb * 512:(nb + 1) * 512],
                                  qT[c * 128:(c + 1) * 128, nb * 512:(nb + 1) * 512])
            qT_t.append(t)
        kN_t = io.tile([128, NKB * 256], F32, tag="kN", name="kN_t")
        for b in range(NKB):
            nc.sync.dma_start(kN_t[:, b * 256:(b + 1) * 256],
                              kN[b * 128:(b + 1) * 128, :])
        qsT_t = [io.tile([128, TPQ], F32, tag=f"qsT{c}", name=f"qsT{c}") for c in range(2)]
        cT_t = [io.tile([128, TPQ], F32, tag=f"cT{c}", name=f"cT{c}") for c in range(2)]
        enhT_t = [io.tile([128, TPQ], F32, tag=f"enhT{c}", name=f"enhT{c}") for c in range(2)]

        # ---- P0: qsT[g, t] = (q @ W_score.T).T ----
        for c in range(2):          # g chunk (psum partition dim)
            for nb in range(4):     # 512-wide t' blocks
                ps = pmm.tile([128, 512], F32, tag="mm", name="ps")
                for h in range(2):  # contraction chunk
                    nc.tensor.matmul(
                        ps[:],
                        wst[h][:, c * 128:(c + 1) * 128],
                        qT_t[h][:, nb * 512:(nb + 1) * 512],
                        start=(h == 0), stop=(h == 1))
                nc.vector.tensor_copy(qsT_t[c][:, nb * 512:(nb + 1) * 512], ps[:])

        # ---- main loop over query tiles ----
        def p1(j):
            # scores[t', s-window 256] + additive band mask, via PSUM accum
            ps = pmm.tile([128, 256], F32, tag="mm", name="ps")
            for c in range(2):
                nc.tensor.matmul(
                    ps[:],
                    qsT_t[c][:, j * 128:(j + 1) * 128],
                    kT_t[c][:, j * 128: j * 128 + 256],
                    start=(c == 0), stop=False)
            nc.tensor.matmul(ps[:], ident[:],
                             (mask_t0 if j == 0 else mask_std)[:],
                             start=False, stop=True)
            # exp (no max subtraction needed) + row sums
            e_t = wk.tile([128, 256], F32, tag="e", name="e_t")
            den = stat.tile([128, 1], F32, tag="den", name="den")
            nc.scalar.activation(e_t[:], ps[:], AF.Exp, accum_out=den[:])
            rec = stat.tile([128, 1], F32, tag="rec", name="rec")
            nc.vector.reciprocal(rec[:], den[:])
            w_t = wk.tile([128, 256], F32, tag="w", name="w_t")
            nc.vector.tensor_scalar_mul(w_t[:], e_t[:], rec[:])
            # transpose w -> [s', t'] for the PV matmul
            pw = pwt.tile([128, 256], F32, tag="pw", name="pw")
            nc.tensor.transpose(pw[:, 0:128], w_t[:, 0:128], ident[:])
            nc.tensor.transpose(pw[:, 128:256], w_t[:, 128:256], ident[:])
            wT_t = wk.tile([128, 256], F32, tag="wT", name="wT_t")
            nc.vector.tensor_copy(wT_t[:], pw[:])
            # cT[h, t'] = sum_s k[s, h] * w[t', s] over the 2 key blocks
            pc = pct.tile([128, 256], F32, tag="pc", name="pc")
            for h in range(2):
                for bi in range(2):
                    blk = j + bi
                    nc.tensor.matmul(
                        pc[:, h * 128:(h + 1) * 128],
                        kN_t[:, blk * 256 + h * 128: blk * 256 + (h + 1) * 128],
                        wT_t[:, bi * 128:(bi + 1) * 128],
                        start=(bi == 0), stop=(bi == 1))
            for h in range(2):
                nc.vector.tensor_copy(cT_t[h][:, j * 128:(j + 1) * 128],
                                      pc[:, h * 128:(h + 1) * 128])

        def p2(nb):
            # enhT[f, t'] = tanh(W_enh.T stacked over [cT, qT] + b_enh)
            rhs_tiles = [cT_t[0], cT_t[1], qT_t[0], qT_t[1]]
            for f in range(2):
                pe_ = pmm.tile([128, 512], F32, tag="mm", name="pe_")
                for d in range(4):
                    nc.tensor.matmul(
                        pe_[:],
                        wet[d][:, f * 128:(f + 1) * 128],
                        rhs_tiles[d][:, nb * 512:(nb + 1) * 512],
                        start=(d == 0), stop=(d == 3))
                nc.scalar.activation(enhT_t[f][:, nb * 512:(nb + 1) * 512],
                                     pe_[:], AF.Tanh, bias=bet[f][:, 0:1])

        def p3(j):
            # z = enh @ W_mask.T + b_mask ; out = sigmoid(z) = 0.5*tanh(z/2)+0.5
            pm = pmm.tile([128, F_OUT], F32, tag="mm", name="pm")
            for f in range(2):
                nc.tensor.matmul(pm[:], enhT_t[f][:, j * 128:(j + 1) * 128],
                                 wmt[f][:], start=(f == 0), stop=False)
            nc.tensor.matmul(pm[:], ones_t[:], bm_t[:], start=False, stop=True)
            o_t = wk.tile([128, F_OUT], F32, tag="o", name="o_t")
            nc.scalar.activation(o_t[:], pm[:], AF.Tanh, scale=0.5)
            o2_t = wk.tile([128, F_OUT], F32, tag="o2", name="o2_t")
            nc.vector.tensor_scalar(o2_t[:], o_t[:], 0.5, 0.5,
                                    op0=ALU.mult, op1=ALU.add)
            rows = min(128, T - j * 128)
            nc.sync.dma_start(out[j * 128: j * 128 + rows, :], o2_t[0:rows, :])

        for j in range(NT):
            p1(j)
            if j % 4 == 3:
                nb = j // 4
                p2(nb)
                for jj in range(nb * 4, nb * 4 + 4):
                    p3(jj)

    return nc


def _prep_shared(W_score, W_enh, b_enh, W_mask, b_mask):
    WsT = np.ascontiguousarray(W_score.T.astype(np.float32))        # [h, g]
    WeT = np.ascontiguousarray(W_enh.T.astype(np.float32))          # [d, f]
    WmT = np.ascontiguousarray(W_mask.T.astype(np.float32))         # [f, o]
    be = np.ascontiguousarray(b_enh.astype(np.float32).reshape(H, 1))
    bm = np.ascontiguousarray(b_mask.astype(np.float32).reshape(1, F_OUT))
    return WsT, WeT, WmT, be, bm


def make_in_maps(k, q, W_score, W_enh, b_enh, W_mask, b_mask):
    k = np.asarray(k, np.float32)
    q = np.asarray(q, np.float32)
    WsT, WeT, WmT, be, bm = _prep_shared(
        np.asarray(W_score, np.float32), np.asarray(W_enh, np.float32),
        np.asarray(b_enh, np.float32), np.asarray(W_mask, np.float32),
        np.asarray(b_mask, np.float32))
    in_maps = []
    for b in range(N_CORES):
        kb = np.zeros((TPK, H), np.float32)
        kb[128:128 + T] = k[b]
        qb = np.zeros((TPQ, H), np.float32)
        qb[:T] = q[b]
        in_maps.append({
            "kT": np.ascontiguousarray(kb.T),
            "kN": kb,
            "qT": np.ascontiguousarray(qb.T),
            "WsT": WsT, "WeT": WeT, "WmT": WmT, "be": be, "bm": bm,
        })
    return in_maps


def get_nc():
    if "nc" not in _CACHE:
        nc = build_nc()
        nc.finalize()
        _CACHE["nc"] = nc
    return _CACHE["nc"]


def kernel(k, q, W_score, W_enh, b_enh, W_mask, b_mask):
    in_maps = make_in_maps(k, q, W_score, W_enh, b_enh, W_mask, b_mask)
    res = run_bass_kernel_spmd(get_nc(), in_maps, list(range(N_CORES)))
    return np.stack([r["out"] for r in res.results], 0)
